# revision 2
# baseline (speedup 1.0000x reference)
"""Trainium2 Bass kernel for nn_AttentionLayer (S=H=4096, fp32), 8-core tensor-parallel.

Sharding: core c owns rows R_c = [c*512, (c+1)*512) of the output.
  - kT_c = (x[R_c] @ Wk.T + bk).T computed locally -> AllGather -> full kT on every core
  - qT_c, v_c computed locally (overlapping the AllGather)
  - scores_c = q_c @ kT * (1/64), softmax rows, out_c = attn_c * v_c
Matmuls run in float32r (TF32-like, ~1.5e-4 rel err) at full PE rate.

kernel(**inputs) takes FULL inputs, shards on host, runs SPMD on cores 0-7 via
run_bass_kernel_spmd, and reassembles the full [4096, 4096] output.
"""
import numpy as np

import concourse.bacc as bacc
import concourse.mybir as mybir
import concourse.tile as tile
from concourse.bass_utils import run_bass_kernel_spmd

S = 4096
H = 4096
NCORES = 8
IB = S // NCORES          # 512 rows per core
JT = H // 128             # 32 contraction tiles
HT = H // 128             # 32 output h-tiles
HC = H // 512             # 8 h-chunks of 512
IT = IB // 128            # 4 i-tiles per core
F32 = mybir.dt.float32
F32R = mybir.dt.float32r
AF = mybir.ActivationFunctionType
SCALE = 1.0 / 64.0        # 1/sqrt(H)


def build_kernel():
    nc = bacc.Bacc(None, target_bir_lowering=False)

    x_ext = nc.declare_dram_parameter("xT", [H, IB], F32, isOutput=False)
    wq_ext = nc.declare_dram_parameter("WqT", [H, H], F32, isOutput=False)
    wk_ext = nc.declare_dram_parameter("WkT", [H, H], F32, isOutput=False)
    wv_ext = nc.declare_dram_parameter("WvT", [H, H], F32, isOutput=False)
    bq_ext = nc.declare_dram_parameter("bqT", [128, HT], F32, isOutput=False)
    bk_ext = nc.declare_dram_parameter("bkT", [128, HT], F32, isOutput=False)
    bv_ext = nc.declare_dram_parameter("bvR", [128, H], F32, isOutput=False)
    out_ext = nc.declare_dram_parameter("out", [IB, H], F32, isOutput=True)

    with tile.TileContext(nc) as tc:
        with (
            tc.tile_pool(name="persist", bufs=1) as persist,
            tc.tile_pool(name="dram", bufs=1, space="DRAM") as dram,
        ):
            # long-lived SBUF
            qT_sb = persist.tile([128, HT, IB], F32R, name="qT_sb")       # 64KB/part
            bq_sb = persist.tile([128, HT], F32, name="bq_sb")
            bk_sb = persist.tile([128, HT], F32, name="bk_sb")
            nc.sync.dma_start(bq_sb[:], bq_ext[:])
            nc.sync.dma_start(bk_sb[:], bk_ext[:])

            # DRAM scratch
            kb_dram = dram.tile([H, IB], F32, name="kb_dram")
            kT_all = dram.tile([NCORES * H, IB], F32, name="kT_all", addr_space="Shared")
            v_dram = dram.tile([IB, H], F32, name="v_dram")

            # ---------------- QKV phases ----------------
            with (
                tc.tile_pool(name="xpool", bufs=1) as xpool,
                tc.tile_pool(name="wpool", bufs=6) as wpool,
                tc.tile_pool(name="spool", bufs=4) as spool,
                tc.tile_pool(name="bvpool", bufs=1) as bvpool,
                tc.tile_pool(name="psA", bufs=2, space="PSUM") as psA,
            ):
                xr = xpool.tile([128, JT, IB], F32R, name="xr")           # 64KB/part
                nc.sync.dma_start(
                    xr[:], x_ext.rearrange("(n p) i -> p n i", p=128).bitcast(F32R)
                )
                bv_sb = bvpool.tile([128, H], F32, name="bv_sb")
                nc.sync.dma_start(bv_sb[:], bv_ext[:])

                def proj_hm(w_ext_t, epilogue):
                    """out[h, i] = sum_j wT[j, h] * xT[j, i]; epilogue(ps, ht)."""
                    for hc in range(HC):
                        ps = [psA.tile([128, IB], F32, tag=f"ps{t}", name=f"ps{t}")
                              for t in range(4)]
                        for j in range(JT):
                            wt = wpool.tile([128, 512], F32R, tag="w", name="wt")
                            eng = nc.sync if j % 2 == 0 else nc.scalar
                            eng.dma_start(
                                wt[:],
                                w_ext_t[j * 128:(j + 1) * 128,
                                        hc * 512:(hc + 1) * 512].bitcast(F32R),
                            )
                            for t in range(4):
                                nc.tensor.matmul(
                                    ps[t][:], wt[:, t * 128:(t + 1) * 128],
                                    xr[:, j, :],
                                    start=(j == 0), stop=(j == JT - 1),
                                )
                        for t in range(4):
                            epilogue(ps[t], hc * 4 + t)

                # K phase: kT_c with bias -> kb_dram
                def k_epi(ps, ht):
                    st = spool.tile([128, IB], F32, tag="stage", name="st")
                    nc.scalar.activation(st[:], ps[:], AF.Identity,
                                         bias=bk_sb[:, ht:ht + 1], scale=1.0)
                    nc.sync.dma_start(kb_dram[ht * 128:(ht + 1) * 128, :], st[:])

                proj_hm(wk_ext, k_epi)

                # AllGather kT (overlaps Q and V phases below)
                nc.gpsimd.collective_compute(
                    "AllGather",
                    mybir.AluOpType.bypass,
                    replica_groups=[list(range(NCORES))],
                    ins=[kb_dram[:].opt()],
                    outs=[kT_all[:].opt()],
                )

                # Q phase: qT_c scaled+biased, fp32r, resident
                def q_epi(ps, ht):
                    nc.scalar.activation(qT_sb[:, ht, :], ps[:], AF.Identity,
                                         bias=bq_sb[:, ht:ht + 1], scale=SCALE)

                proj_hm(wq_ext, q_epi)

                # V phase: v_c[i, h] = sum_j xT[j, i] * wvT[j, h] + bv -> v_dram
                for hc in range(HC):
                    ps = [psA.tile([128, 512], F32, tag=f"ps{t}", name=f"ps{t}")
                          for t in range(4)]
                    for j in range(JT):
                        wt = wpool.tile([128, 512], F32R, tag="w", name="wt")
                        eng = nc.sync if j % 2 == 0 else nc.scalar
                        eng.dma_start(
                            wt[:],
                            wv_ext[j * 128:(j + 1) * 128,
                                   hc * 512:(hc + 1) * 512].bitcast(F32R),
                        )
                        for it in range(IT):
                            nc.tensor.matmul(
                                ps[it][:], xr[:, j, it * 128:(it + 1) * 128], wt[:],
                                start=(j == 0), stop=(j == JT - 1),
                            )
                    for it in range(IT):
                        st = spool.tile([128, 512], F32, tag="stage", name="st")
                        nc.vector.tensor_add(st[:], ps[it][:],
                                             bv_sb[:, hc * 512:(hc + 1) * 512])
                        nc.sync.dma_start(
                            v_dram[it * 128:(it + 1) * 128,
                                   hc * 512:(hc + 1) * 512], st[:])

            # ---------------- scores + softmax + mix ----------------
            with (
                tc.tile_pool(name="scpool", bufs=1) as scpool,
                tc.tile_pool(name="ktpool", bufs=6) as ktpool,
                tc.tile_pool(name="vpool", bufs=2) as vpool,
                tc.tile_pool(name="smpool", bufs=1) as smpool,
                tc.tile_pool(name="psB", bufs=2, space="PSUM") as psB,
            ):
                scores = [scpool.tile([128, S], F32, name=f"scores{it}")
                          for it in range(IT)]
                sums_parts = smpool.tile([128, IT, NCORES], F32, name="sums_parts")
                sums = smpool.tile([128, IT], F32, name="sums")
                recip = smpool.tile([128, IT], F32, name="recip")

                for r in range(NCORES):
                    ps = [psB.tile([128, 512], F32, tag=f"pb{it}", name=f"pb{it}")
                          for it in range(IT)]
                    for ht in range(HT):
                        kt = ktpool.tile([128, 512], F32R, tag="kt", name="kt")
                        base = r * H + ht * 128
                        eng = nc.sync if ht % 2 == 0 else nc.scalar
                        eng.dma_start(kt[:], kT_all[base:base + 128, :].bitcast(F32R))
                        for it in range(IT):
                            nc.tensor.matmul(
                                ps[it][:], qT_sb[:, ht, it * 128:(it + 1) * 128],
                                kt[:],
                                start=(ht == 0), stop=(ht == HT - 1),
                            )
                    for it in range(IT):
                        nc.scalar.activation(
                            scores[it][:, r * 512:(r + 1) * 512], ps[it][:], AF.Exp,
                            accum_out=sums_parts[:, it, r:r + 1],
                        )

                for it in range(IT):
                    nc.vector.reduce_sum(sums[:, it:it + 1], sums_parts[:, it, :],
                                         axis=mybir.AxisListType.X)
                    nc.vector.reciprocal(recip[:, it:it + 1], sums[:, it:it + 1])
                    vt = vpool.tile([128, H], F32, tag="vt", name="vt")
                    nc.sync.dma_start(vt[:], v_dram[it * 128:(it + 1) * 128, :])
                    nc.vector.tensor_mul(scores[it][:], scores[it][:], vt[:])
                    nc.vector.tensor_scalar_mul(scores[it][:], scores[it][:],
                                                recip[:, it:it + 1])
                    nc.sync.dma_start(out_ext[it * 128:(it + 1) * 128, :],
                                        scores[it][:])

    nc.compile()
    return nc


_NC_CACHE = None


def _get_nc():
    global _NC_CACHE
    if _NC_CACHE is None:
        _NC_CACHE = build_kernel()
    return _NC_CACHE


def prep_inputs(x, Wq, bq, Wk, bk, Wv, bv):
    """Host-side shard prep. Returns in_maps for the 8 cores."""
    x = np.asarray(x, dtype=np.float32)
    xT = np.ascontiguousarray(x.T)                      # [H, S]
    WqT = np.ascontiguousarray(np.asarray(Wq, np.float32).T)  # [j, h]
    WkT = np.ascontiguousarray(np.asarray(Wk, np.float32).T)
    WvT = np.ascontiguousarray(np.asarray(Wv, np.float32).T)
    bqT = np.ascontiguousarray((np.asarray(bq, np.float32) * SCALE).reshape(HT, 128).T)
    bkT = np.ascontiguousarray(np.asarray(bk, np.float32).reshape(HT, 128).T)
    bvR = np.ascontiguousarray(
        np.broadcast_to(np.asarray(bv, np.float32), (128, H)))
    in_maps = []
    for c in range(NCORES):
        in_maps.append({
            "xT": np.ascontiguousarray(xT[:, c * IB:(c + 1) * IB]),
            "WqT": WqT, "WkT": WkT, "WvT": WvT,
            "bqT": bqT, "bkT": bkT, "bvR": bvR,
        })
    return in_maps


def kernel(x, Wq, bq, Wk, bk, Wv, bv):
    nc = _get_nc()
    in_maps = prep_inputs(x, Wq, bq, Wk, bk, Wv, bv)
    res = run_bass_kernel_spmd(nc, in_maps, core_ids=list(range(NCORES)))
    return np.concatenate([res.results[c]["out"] for c in range(NCORES)], axis=0)
